# revision 16
# baseline (speedup 1.0000x reference)
"""GQA causal-attention prefill kernel for Trainium2, tensor-parallel over 8 NeuronCores.

Reference semantics: q/k/v projections + RoPE + causal GQA attention +
output projection, B=2, T=2048, D=4096, 32 q heads, 8 kv heads, head_dim 128.

Sharding: head-parallel. Core c gets q heads [4c, 4c+4), kv head c, and the
matching wo slice; each core computes a full-shape partial output
o_part = attn(heads of c) @ wo_c and the host sums the 8 partials.

v2 design (vs the 1250us baseline):
  - all matmul operands bf16 (same PE rate as fp32r at 512-wide, but full
    rate at ANY width, half the DMA/SBUF, and no walrus f32r-producer
    quirks). PSUM accumulation stays fp32; rope/softmax math in fp32.
  - ONE phase-1 sweep over all 8 token chunks (both batches): weights are
    loaded once (no per-batch reload) and no rope tail is exposed at a
    phase boundary.
  - weight DMAs are k-slice-interleaved with chunk-0 x tiles so the first
    projection matmul starts after ~1.5MB instead of ~15MB of DMA.
  - natural-layout AV with a ones column appended to v: one accumulation
    yields both sum_s p*v AND the softmax denominator l (column 128),
    removing the baseline's l-matmul + broadcast matmul (~96us of PE) and
    its [128,512] reciprocal/multiply DVE work. The denominator lands
    per-partition, so normalization fuses into the ACT eviction as a
    per-partition scale.
  - causal masking by construction: fully-masked 128-col blocks are never
    computed (scores matmuls cover only the valid column range; AV matmuls
    for fully-invalid blocks are skipped). Only the diagonal [128,128]
    triangles get an in-place 0/1 mask multiply on DVE.
  - transposes (v to natural layout, attention-out to head-major) run on
    the DMA XBAR (16-bit SBUF->SBUF transpose), not the PE.
  - o-projection groups of chunk N interleave into chunk N+1's scores/exp
    stage, where the PE would otherwise wait on ACT exps.
  - rope combine runs on DVE (bf16 writes), not the slow GpSimd.
"""

import os
import sys

sys.path.insert(0, "/opt/trn_rl_repo")

import numpy as np

B = 2
T = 2048
TOK = B * T
D = 4096
NQ = 32
NKV = 8
H = 128
HH = H // 2
THETA = 10000.0
NCORES = 8
NHC = NQ // NCORES          # q heads per core (4)
KPC = D // H                # contraction chunks of 128 over D (32)
TCH = 512                   # token chunk
NCH = TOK // TCH            # 8 chunks across both batches
NTCH = T // TCH             # 4 chunks per batch
NSUB = TCH // H             # 4 128-token subtiles per chunk
C_SM = 1.0 / np.sqrt(H)     # softmax scale


def _build_bass():
    import concourse.bacc as bacc
    import concourse.mybir as mybir
    import concourse.tile as tile
    from concourse.masks import make_identity

    f32 = mybir.dt.float32
    bf16 = mybir.dt.bfloat16
    Exp = mybir.ActivationFunctionType.Exp
    Copy = mybir.ActivationFunctionType.Copy

    nc = bacc.Bacc("TRN2", target_bir_lowering=False, debug=False,
                   num_devices=NCORES)

    xT = nc.declare_dram_parameter("xT", [D, TOK], bf16, isOutput=False)
    wq = nc.declare_dram_parameter("wq", [NHC, D, H], bf16, isOutput=False)
    wk = nc.declare_dram_parameter("wk", [D, H], bf16, isOutput=False)
    wv = nc.declare_dram_parameter("wv", [D, H], bf16, isOutput=False)
    wo = nc.declare_dram_parameter("wo", [NHC, H, D], bf16, isOutput=False)
    # rope tables duplicated across both partition halves (row p and p+64
    # hold the same values); one batch's worth - positions are identical
    # across batches.
    cosT = nc.declare_dram_parameter("cosT", [H, T], f32, isOutput=False)
    sinT = nc.declare_dram_parameter("sinT", [H, T], f32, isOutput=False)
    o_part = nc.declare_dram_parameter("o_part", [TOK, D], f32, isOutput=True)

    with tile.TileContext(nc) as tc:
        from contextlib import ExitStack

        with ExitStack() as top:
            consts = top.enter_context(tc.tile_pool(name="consts", bufs=1))
            # causal triangle mask (same [128,128] wedge for every diagonal
            # block): tri[s, c] = 1 iff c >= s
            tri_f32 = consts.tile([H, H], f32, tag="trif")
            nc.vector.memset(tri_f32, 1.0)
            nc.gpsimd.affine_select(
                out=tri_f32, in_=tri_f32,
                compare_op=mybir.AluOpType.is_ge,
                fill=0.0, base=0,
                pattern=[[1, H]],
                channel_multiplier=-1,
            )
            tri = consts.tile([H, H], bf16, tag="tri")
            nc.vector.tensor_copy(tri, tri_f32)
            # bf16 identity for PE transposes of bf16 tiles
            ident_f32 = consts.tile([H, H], f32, tag="idf")
            make_identity(nc, ident_f32)
            ident = consts.tile([H, H], bf16, tag="id")
            nc.vector.tensor_copy(ident, ident_f32)

            # persistent activations for both batches
            act = top.enter_context(tc.tile_pool(name="act", bufs=1))
            qTs = [act.tile([H, NHC, TCH], bf16, tag=f"qT{i}", name=f"qT{i}")
                   for i in range(NCH)]
            kTs = [act.tile([H, TCH], bf16, tag=f"kT{i}", name=f"kT{i}")
                   for i in range(NCH)]
            # v natural [s, j, col]: col 0:128 = v, col 128 = 1.0 (the ones
            # column that accumulates the softmax denominator in AV),
            # col 129 = 0 pad for 4-byte row alignment
            vs = [act.tile([H, NSUB, H + 2], bf16, tag=f"v{i}", name=f"v{i}")
                  for i in range(NCH)]
            wopool = top.enter_context(tc.tile_pool(name="wopool", bufs=1))
            wo_sb = wopool.tile([H, NHC, D], bf16, tag="wo")

            # ---------------- phase 1: projections + rope, one sweep ------
            with ExitStack() as ph1:
                wpool = ph1.enter_context(tc.tile_pool(name="wpool", bufs=1))
                xpool = ph1.enter_context(tc.tile_pool(name="xpool", bufs=6))
                rtmp = ph1.enter_context(tc.tile_pool(name="rtmp", bufs=2))
                pj = ph1.enter_context(
                    tc.tile_pool(name="pj", bufs=1, space="PSUM"))
                pv = ph1.enter_context(
                    tc.tile_pool(name="pv", bufs=2, space="PSUM"))

                wq_src = wq.rearrange("h (c p) m -> p h c m", p=H)
                wk_src = wk.rearrange("(c p) m -> p c m", p=H)
                wv_src = wv.rearrange("(c p) m -> p c m", p=H)
                wqs = [wpool.tile([H, KPC, H], bf16, tag=f"wq{i}",
                                  name=f"wq{i}") for i in range(NHC)]
                wk_sb = wpool.tile([H, KPC, H], bf16, tag="wk")
                wv_sb = wpool.tile([H, KPC, H], bf16, tag="wv")
                cos_sb = wpool.tile([H, T], f32, tag="cos")
                sin_sb = wpool.tile([H, T], f32, tag="sin")
                # k-slice-interleaved weight+x loads: the first matmul needs
                # only wq0 slice c8=0 and x chunk-0 k=0, so issue those first
                x_c0 = []
                for c8 in range(4):
                    sl = slice(c8 * 8, (c8 + 1) * 8)
                    for i in range(NHC):
                        nc.sync.dma_start(out=wqs[i][:, sl, :],
                                          in_=wq_src[:, i, sl, :])
                    nc.sync.dma_start(out=wk_sb[:, sl, :], in_=wk_src[:, sl, :])
                    nc.sync.dma_start(out=wv_sb[:, sl, :], in_=wv_src[:, sl, :])
                    for k in range(c8 * 8, (c8 + 1) * 8):
                        x_t = xpool.tile([H, TCH], bf16, tag="x")
                        nc.sync.dma_start(out=x_t,
                                          in_=xT[k * H:(k + 1) * H, 0:TCH])
                        x_c0.append(x_t)
                nc.sync.dma_start(out=cos_sb, in_=cosT[:, 0:T])
                nc.sync.dma_start(out=sin_sb, in_=sinT[:, 0:T])

                def rope_math(direct, swap, dst_first, dst_second, cs, sn,
                              eng):
                    # q'[0:64] = q[0:64]*cos - q[64:]*sin
                    # q'[64:]  = q[64:]*cos + q[0:64]*sin
                    # The rope tables are duplicated across partition halves
                    # precisely so both muls run as single [128,512] ops.
                    # (both-SBUF operand pairs must share a base partition,
                    # hence the half-swapped staging copy `swap`.)
                    tmp = rtmp.tile([H, TCH], f32, tag="rt", bufs=2)
                    tmp2 = rtmp.tile([H, TCH], f32, tag="rt2", bufs=2)
                    eng.tensor_mul(tmp, swap, sn)
                    eng.tensor_mul(tmp2, direct, cs)
                    eng.tensor_sub(dst_first, tmp2[0:HH, :], tmp[0:HH, :])
                    eng.tensor_add(dst_second, tmp2[HH:H, :], tmp[HH:H, :])

                last = KPC - 1
                for tch in range(NCH):
                    t0 = tch * TCH
                    tl = (tch % NTCH) * TCH      # token offset within batch
                    g_ps = [pj.tile([H, TCH], f32, tag=f"g{i}",
                                    name=f"g_ps{i}") for i in range(6)]
                    for k in range(KPC):
                        if tch == 0:
                            x_t = x_c0[k]
                        else:
                            x_t = xpool.tile([H, TCH], bf16, tag="x")
                            nc.sync.dma_start(
                                out=x_t,
                                in_=xT[k * H:(k + 1) * H, t0:t0 + TCH])
                        lhs = [wqs[0][:, k, :], wqs[1][:, k, :],
                               wqs[2][:, k, :], wqs[3][:, k, :],
                               wk_sb[:, k, :], wv_sb[:, k, :]]
                        for i in range(6):
                            nc.tensor.matmul(
                                g_ps[i], lhs[i], x_t,
                                start=(k == 0), stop=(k == last),
                                skip_group_check=True)
                    # wo prefetch once the DMA queues have drained the bulk
                    # of the weight traffic; lands well before phase 2.
                    if tch == 5:
                        wo_src = wo.rearrange("h p d -> p h d")
                        for dc8 in range(8):
                            sl = slice(dc8 * TCH, (dc8 + 1) * TCH)
                            nc.sync.dma_start(out=wo_sb[:, :, sl],
                                              in_=wo_src[:, :, sl])
                    cs = cos_sb[:, tl:tl + TCH]
                    sn = sin_sb[:, tl:tl + TCH]
                    # v staging evicts FIRST on ACT: the PE's next work (the
                    # v transposes) waits only on this copy, not on the five
                    # rope staging copies behind it.
                    vstage = rtmp.tile([H, TCH], bf16, tag="vstage", bufs=2)
                    nc.scalar.activation(vstage, g_ps[5], Copy)
                    # rope staging: every bank's copies are emitted before
                    # any rope math so banks free quickly for the next
                    # chunk's accumulation groups. Staging is 5-deep so no
                    # release ever queues behind another group's rope math.
                    rel = []
                    for g in range(5):
                        direct = rtmp.tile([H, TCH], f32, tag="rdir",
                                           bufs=5, name="direct")
                        swap = rtmp.tile([H, TCH], f32, tag="rswap",
                                         bufs=5, name="swap")
                        nc.scalar.activation(direct, g_ps[g], Copy)
                        nc.vector.tensor_copy(swap[0:HH, :], g_ps[g][HH:H, :])
                        nc.vector.tensor_copy(swap[HH:H, :], g_ps[g][0:HH, :])
                        rel.append((direct, swap))
                    # v: PE-transpose each [128,128] block into natural
                    # layout; ones column via memset (tiles are written once)
                    nc.vector.memset(vs[tch][:, :, H:H + 2], 0.0)
                    nc.vector.memset(vs[tch][:, :, H:H + 1], 1.0)
                    for j in range(NSUB):
                        tp = pv.tile([H, H], bf16, tag="vtp")
                        nc.tensor.transpose(
                            tp, vstage[:, j * H:(j + 1) * H], ident)
                        nc.vector.tensor_copy(vs[tch][:, j, 0:H], tp)
                    # the last two chunks' rope runs on the otherwise-idle
                    # GpSimd: their q/k are consumed late in phase 2, and
                    # keeping them off DVE means phase 2's first tri-masks
                    # and reciprocals don't queue behind a rope backlog.
                    eng = nc.gpsimd if tch >= NCH - 2 else nc.vector
                    rope_math(*rel[4], kTs[tch][0:HH, :], kTs[tch][HH:H, :],
                              cs, sn, eng)
                    for i in range(NHC):
                        rope_math(*rel[i], qTs[tch][0:HH, i, :],
                                  qTs[tch][HH:H, i, :], cs, sn, eng)

            # ---------------- phase 2: attention + o-projection -----------
            with ExitStack() as ph2:
                ppool = ph2.enter_context(tc.tile_pool(name="ppool", bufs=2))
                otpool = ph2.enter_context(tc.tile_pool(name="otpool", bufs=2))
                small = ph2.enter_context(tc.tile_pool(name="small", bufs=4))
                opool = ph2.enter_context(tc.tile_pool(name="opool", bufs=8))
                oscr = ph2.enter_context(
                    tc.tile_pool(name="oscr", bufs=8, space="DRAM"))
                ps_s = ph2.enter_context(
                    tc.tile_pool(name="ps_s", bufs=3, space="PSUM"))
                ps_av = ph2.enter_context(
                    tc.tile_pool(name="ps_av", bufs=2, space="PSUM"))
                ps_o = ph2.enter_context(
                    tc.tile_pool(name="ps_o", bufs=2, space="PSUM"))

                def oproj_group(b_, qc_, outT_prev, g):
                    # group g = u*8 + dc of the 32 o-projection groups for
                    # q-chunk (b_, qc_); eviction on DVE (ACT is saturated
                    # by exps while these interleave into the scores stage)
                    u, dc = divmod(g, 8)
                    trow = b_ * T + qc_ * TCH + u * H
                    ops = ps_o.tile([H, TCH], f32, tag="o")
                    for hh in range(NHC):
                        nc.tensor.matmul(
                            ops,
                            outT_prev[hh][u],
                            wo_sb[:, hh, dc * TCH:(dc + 1) * TCH],
                            start=(hh == 0), stop=(hh == NHC - 1),
                            skip_group_check=True)
                    o_sb = opool.tile([H, TCH], f32, tag="osb")
                    nc.vector.tensor_copy(o_sb, ops)
                    # split across two queues: a single [128,512] f32 write
                    # is 128 x 2KB descriptors on one queue (~8us latency),
                    # which stalled the o_sb/ps_o recycling chain
                    nc.sync.dma_start(
                        out=o_part[trow:trow + HH, dc * TCH:(dc + 1) * TCH],
                        in_=o_sb[0:HH, :])
                    nc.sync.dma_start(
                        out=o_part[trow + HH:trow + H,
                                   dc * TCH:(dc + 1) * TCH],
                        in_=o_sb[HH:H, :])

                pending = None   # (b, qc, outT_tiles) awaiting o-projection
                for b in range(B):
                    for qc in range(NTCH):
                        n_st = (qc + 1) * NSUB
                        # per-(h,u) [128,128] head-major tiles; they must be
                        # contiguous whole tiles because the XBAR transpose
                        # writes garbage to strided destinations on hardware
                        outT_sb = [[otpool.tile([H, H], bf16,
                                                tag=f"ot{hh}_{uu}",
                                                name=f"ot{hh}_{uu}")
                                    for uu in range(NSUB)]
                                   for hh in range(NHC)]
                        for h in range(NHC):
                            rhs_q = qTs[NTCH * b + qc][:, h, :]
                            # stage A: scores + exp for all s-tiles, with the
                            # previous chunk's o-proj groups interleaved to
                            # keep the PE busy while ACT works through exps
                            fill_done = 0
                            pT2s = []
                            for st in range(n_st):
                                j = st - qc * NSUB   # >=0: diagonal band
                                c0 = max(j, 0) * H
                                sps = ps_s.tile([H, TCH], f32, tag="s")
                                pt = ppool.tile([H, TCH], bf16, tag=f"p{st}",
                                                name=f"p{st}")
                                pT2s.append(pt)
                                kt = kTs[NTCH * b + st // NSUB][
                                    :, (st % NSUB) * H:(st % NSUB + 1) * H]
                                nc.tensor.matmul(sps[:, c0:TCH], kt,
                                                 rhs_q[:, c0:TCH],
                                                 start=True, stop=True)
                                nc.scalar.activation(
                                    pt[:, c0:TCH], sps[:, c0:TCH],
                                    Exp, scale=C_SM)
                                if j >= 0:
                                    # in-place 0/1 triangle on the diagonal
                                    nc.vector.tensor_mul(
                                        pt[:, c0:c0 + H],
                                        pt[:, c0:c0 + H], tri)
                                if pending is not None:
                                    want = (st + 1) * 8 // n_st
                                    while fill_done < want:
                                        oproj_group(*pending[:3],
                                                    h * 8 + fill_done)
                                        fill_done += 1
                            if pending is not None and h == NHC - 1:
                                pending = None
                            # stage B: AV per 128-token subtile; the ones
                            # column of v accumulates the denominator into
                            # col 128 of the same PSUM group
                            for u in range(NSUB):
                                st_hi = min(n_st - 1, qc * NSUB + u)
                                avp = ps_av.tile([H, TCH], f32, tag="av")
                                for st in range(st_hi + 1):
                                    nc.tensor.matmul(
                                        avp[:, 0:H + 2],
                                        pT2s[st][:, u * H:(u + 1) * H],
                                        vs[NTCH * b + st // NSUB][
                                            :, st % NSUB, :],
                                        start=(st == 0), stop=(st == st_hi),
                                        skip_group_check=True)
                                recip = small.tile([H, 1], f32, tag="rc")
                                nc.vector.reciprocal(recip, avp[:, H:H + 1])
                                onat = small.tile([H, H], bf16, tag="on")
                                nc.scalar.mul(onat, avp[:, 0:H], recip)
                                # head-major transpose via DRAM roundtrip on
                                # the XBAR (2 DMA hops, zero engine time)
                                scr = oscr.tile([H, H], bf16, tag="os")
                                nc.sync.dma_start(out=scr, in_=onat)
                                nc.sync.dma_start_transpose(
                                    out=outT_sb[h][u], in_=scr)
                        pending = (b, qc, outT_sb)
                for g in range(32):
                    oproj_group(*pending[:3], g)

    nc.compile()
    return nc


_NC_CACHE = None


def kernel(x, wq, wk, wv, wo, positions):
    global _NC_CACHE
    import ml_dtypes
    from concourse.bass_utils import run_bass_kernel_spmd

    bf = ml_dtypes.bfloat16
    x = np.asarray(x, dtype=np.float32)
    positions = np.asarray(positions)

    xT = np.ascontiguousarray(x.reshape(TOK, D).T.astype(bf))
    wq_b = np.asarray(wq, dtype=np.float32).astype(bf)
    wk_b = np.asarray(wk, dtype=np.float32).astype(bf)
    wv_b = np.asarray(wv, dtype=np.float32).astype(bf)
    wo_b = np.asarray(wo, dtype=np.float32).astype(bf)

    # rope tables, transposed [H/2, T], duplicated across partition halves;
    # positions are identical across batches so one batch's worth suffices.
    fraction = 2.0 * np.arange(HH, dtype=np.float32) / H
    timescale = (THETA ** fraction).astype(np.float32)
    pos = positions.reshape(TOK)[:T].astype(np.float32)
    sinusoid = pos[None, :] / timescale[:, None]
    cosT = np.cos(sinusoid).astype(np.float32)
    sinT = np.sin(sinusoid).astype(np.float32)
    cosT = np.ascontiguousarray(np.concatenate([cosT, cosT], axis=0))
    sinT = np.ascontiguousarray(np.concatenate([sinT, sinT], axis=0))

    if _NC_CACHE is None:
        _NC_CACHE = _build_bass()
    nc = _NC_CACHE

    in_maps = []
    for c in range(NCORES):
        in_maps.append({
            "xT": xT,
            "wq": np.ascontiguousarray(wq_b[c * NHC:(c + 1) * NHC]),
            "wk": np.ascontiguousarray(wk_b[c]),
            "wv": np.ascontiguousarray(wv_b[c]),
            "wo": np.ascontiguousarray(wo_b[c * NHC:(c + 1) * NHC]),
            "cosT": cosT,
            "sinT": sinT,
        })

    trace = os.environ.get("BASS_KERNEL_TRACE", "0") == "1"
    res = run_bass_kernel_spmd(nc, in_maps, list(range(NCORES)), trace=trace)
    global LAST_RESULTS
    LAST_RESULTS = res
    out = np.zeros((TOK, D), dtype=np.float32)
    for c in range(NCORES):
        out += res.results[c]["o_part"]
    return out.reshape(B, T, D)


LAST_RESULTS = None


# revision 22
# speedup vs baseline: 1.5592x; 1.5592x over previous
"""GQA causal-attention prefill kernel for Trainium2, tensor-parallel over 8 NeuronCores.

Reference semantics: q/k/v projections + RoPE + causal GQA attention +
output projection, B=2, T=2048, D=4096, 32 q heads, 8 kv heads, head_dim 128.

Sharding: head-parallel. Core c gets q heads [4c, 4c+4), kv head c, and the
matching wo slice; each core computes a full-shape partial output
o_part = attn(heads of c) @ wo_c and the host sums the 8 partials.

v2 design (vs the 1250us baseline):
  - all matmul operands bf16 (same PE rate as fp32r at 512-wide, but full
    rate at ANY width, half the DMA/SBUF, and no walrus f32r-producer
    quirks). PSUM accumulation stays fp32; rope/softmax math in fp32.
  - ONE phase-1 sweep over all 8 token chunks (both batches): weights are
    loaded once (no per-batch reload) and no rope tail is exposed at a
    phase boundary.
  - weight DMAs are k-slice-interleaved with chunk-0 x tiles so the first
    projection matmul starts after ~1.5MB instead of ~15MB of DMA.
  - natural-layout AV with a ones column appended to v: one accumulation
    yields both sum_s p*v AND the softmax denominator l (column 128),
    removing the baseline's l-matmul + broadcast matmul (~96us of PE) and
    its [128,512] reciprocal/multiply DVE work. The denominator lands
    per-partition, so normalization fuses into the ACT eviction as a
    per-partition scale.
  - causal masking by construction: fully-masked 128-col blocks are never
    computed (scores matmuls cover only the valid column range; AV matmuls
    for fully-invalid blocks are skipped). Only the diagonal [128,128]
    triangles get an in-place 0/1 mask multiply on DVE.
  - transposes (v to natural layout, attention-out to head-major) run on
    the DMA XBAR (16-bit SBUF->SBUF transpose), not the PE.
  - o-projection groups of chunk N interleave into chunk N+1's scores/exp
    stage, where the PE would otherwise wait on ACT exps.
  - rope combine runs on DVE (bf16 writes), not the slow GpSimd.
"""

import os
import sys

sys.path.insert(0, "/opt/trn_rl_repo")

import numpy as np

B = 2
T = 2048
TOK = B * T
D = 4096
NQ = 32
NKV = 8
H = 128
HH = H // 2
THETA = 10000.0
NCORES = 8
NHC = NQ // NCORES          # q heads per core (4)
KPC = D // H                # contraction chunks of 128 over D (32)
TCH = 512                   # token chunk
NCH = TOK // TCH            # 8 chunks across both batches
NTCH = T // TCH             # 4 chunks per batch
NSUB = TCH // H             # 4 128-token subtiles per chunk
C_SM = 1.0 / np.sqrt(H)     # softmax scale


def _build_bass():
    import concourse.bacc as bacc
    import concourse.mybir as mybir
    import concourse.tile as tile
    from concourse.masks import make_identity

    f32 = mybir.dt.float32
    bf16 = mybir.dt.bfloat16
    Exp = mybir.ActivationFunctionType.Exp
    Copy = mybir.ActivationFunctionType.Copy

    nc = bacc.Bacc("TRN2", target_bir_lowering=False, debug=False,
                   num_devices=NCORES)

    xT = nc.declare_dram_parameter("xT", [D, TOK], bf16, isOutput=False)
    wq = nc.declare_dram_parameter("wq", [NHC, D, H], bf16, isOutput=False)
    wk = nc.declare_dram_parameter("wk", [D, H], bf16, isOutput=False)
    wv = nc.declare_dram_parameter("wv", [D, H], bf16, isOutput=False)
    wo = nc.declare_dram_parameter("wo", [NHC, H, D], bf16, isOutput=False)
    # rope tables duplicated across both partition halves (row p and p+64
    # hold the same values); one batch's worth - positions are identical
    # across batches.
    cosT = nc.declare_dram_parameter("cosT", [H, T], f32, isOutput=False)
    sinT = nc.declare_dram_parameter("sinT", [H, T], f32, isOutput=False)
    o_part = nc.declare_dram_parameter("o_part", [TOK, D], f32, isOutput=True)

    with tile.TileContext(nc) as tc:
        from contextlib import ExitStack

        with ExitStack() as top:
            consts = top.enter_context(tc.tile_pool(name="consts", bufs=1))
            # causal triangle mask (same [128,128] wedge for every diagonal
            # block): tri[s, c] = 1 iff c >= s
            tri_f32 = consts.tile([H, H], f32, tag="trif")
            nc.vector.memset(tri_f32, 1.0)
            nc.gpsimd.affine_select(
                out=tri_f32, in_=tri_f32,
                compare_op=mybir.AluOpType.is_ge,
                fill=0.0, base=0,
                pattern=[[1, H]],
                channel_multiplier=-1,
            )
            tri = consts.tile([H, H], bf16, tag="tri")
            nc.vector.tensor_copy(tri, tri_f32)
            # bf16 identity for PE transposes of bf16 tiles
            ident_f32 = consts.tile([H, H], f32, tag="idf")
            make_identity(nc, ident_f32)
            ident = consts.tile([H, H], bf16, tag="id")
            nc.vector.tensor_copy(ident, ident_f32)

            # persistent activations for both batches
            act = top.enter_context(tc.tile_pool(name="act", bufs=1))
            qTs = [act.tile([H, NHC, TCH], bf16, tag=f"qT{i}", name=f"qT{i}")
                   for i in range(NCH)]
            kTs = [act.tile([H, TCH], bf16, tag=f"kT{i}", name=f"kT{i}")
                   for i in range(NCH)]
            # v natural [s, j, col]: col 0:128 = v, col 128 = 1.0 (the ones
            # column that accumulates the softmax denominator in AV),
            # col 129 = 0 pad for 4-byte row alignment
            vs = [act.tile([H, NSUB, H + 2], bf16, tag=f"v{i}", name=f"v{i}")
                  for i in range(NCH)]
            wopool = top.enter_context(tc.tile_pool(name="wopool", bufs=1))
            wo_sb = wopool.tile([H, NHC, D], bf16, tag="wo")

            # ---------------- phase 1: projections + rope, one sweep ------
            with ExitStack() as ph1:
                wpool = ph1.enter_context(tc.tile_pool(name="wpool", bufs=1))
                xpool = ph1.enter_context(tc.tile_pool(name="xpool", bufs=6))
                rtmp = ph1.enter_context(tc.tile_pool(name="rtmp", bufs=2))
                pj = ph1.enter_context(
                    tc.tile_pool(name="pj", bufs=1, space="PSUM"))
                pv = ph1.enter_context(
                    tc.tile_pool(name="pv", bufs=2, space="PSUM"))

                wq_src = wq.rearrange("h (c p) m -> p h c m", p=H)
                wk_src = wk.rearrange("(c p) m -> p c m", p=H)
                wv_src = wv.rearrange("(c p) m -> p c m", p=H)
                wqs = [wpool.tile([H, KPC, H], bf16, tag=f"wq{i}",
                                  name=f"wq{i}") for i in range(NHC)]
                wk_sb = wpool.tile([H, KPC, H], bf16, tag="wk")
                wv_sb = wpool.tile([H, KPC, H], bf16, tag="wv")
                cos_sb = wpool.tile([H, T], f32, tag="cos")
                sin_sb = wpool.tile([H, T], f32, tag="sin")
                # k-slice-interleaved weight+x loads: the first matmul needs
                # only wq0 slice c8=0 and x chunk-0 k=0, so issue those first
                x_c0 = []
                for c8 in range(4):
                    sl = slice(c8 * 8, (c8 + 1) * 8)
                    for i in range(NHC):
                        nc.sync.dma_start(out=wqs[i][:, sl, :],
                                          in_=wq_src[:, i, sl, :])
                    nc.sync.dma_start(out=wk_sb[:, sl, :], in_=wk_src[:, sl, :])
                    nc.sync.dma_start(out=wv_sb[:, sl, :], in_=wv_src[:, sl, :])
                    for k in range(c8 * 8, (c8 + 1) * 8):
                        x_t = xpool.tile([H, TCH], bf16, tag="x")
                        nc.sync.dma_start(out=x_t,
                                          in_=xT[k * H:(k + 1) * H, 0:TCH])
                        x_c0.append(x_t)
                nc.sync.dma_start(out=cos_sb, in_=cosT[:, 0:T])
                nc.sync.dma_start(out=sin_sb, in_=sinT[:, 0:T])

                def rope_math(direct, swap, dst_first, dst_second, cs, sn,
                              eng):
                    # q'[0:64] = q[0:64]*cos - q[64:]*sin
                    # q'[64:]  = q[64:]*cos + q[0:64]*sin
                    # The rope tables are duplicated across partition halves
                    # precisely so both muls run as single [128,512] ops.
                    # (both-SBUF operand pairs must share a base partition,
                    # hence the half-swapped staging copy `swap`.)
                    tmp = rtmp.tile([H, TCH], f32, tag="rt", bufs=2)
                    tmp2 = rtmp.tile([H, TCH], f32, tag="rt2", bufs=2)
                    eng.tensor_mul(tmp, swap, sn)
                    eng.tensor_mul(tmp2, direct, cs)
                    eng.tensor_sub(dst_first, tmp2[0:HH, :], tmp[0:HH, :])
                    eng.tensor_add(dst_second, tmp2[HH:H, :], tmp[HH:H, :])

                last = KPC - 1
                for tch in range(NCH):
                    t0 = tch * TCH
                    tl = (tch % NTCH) * TCH      # token offset within batch
                    g_ps = [pj.tile([H, TCH], f32, tag=f"g{i}",
                                    name=f"g_ps{i}") for i in range(6)]
                    for k in range(KPC):
                        if tch == 0:
                            x_t = x_c0[k]
                        else:
                            x_t = xpool.tile([H, TCH], bf16, tag="x")
                            nc.sync.dma_start(
                                out=x_t,
                                in_=xT[k * H:(k + 1) * H, t0:t0 + TCH])
                        lhs = [wqs[0][:, k, :], wqs[1][:, k, :],
                               wqs[2][:, k, :], wqs[3][:, k, :],
                               wk_sb[:, k, :], wv_sb[:, k, :]]
                        for i in range(6):
                            nc.tensor.matmul(
                                g_ps[i], lhs[i], x_t,
                                start=(k == 0), stop=(k == last),
                                skip_group_check=True)
                    # wo prefetch once the DMA queues have drained the bulk
                    # of the weight traffic; lands well before phase 2.
                    if tch == 5:
                        wo_src = wo.rearrange("h p d -> p h d")
                        for dc8 in range(8):
                            sl = slice(dc8 * TCH, (dc8 + 1) * TCH)
                            nc.sync.dma_start(out=wo_sb[:, :, sl],
                                              in_=wo_src[:, :, sl])
                    cs = cos_sb[:, tl:tl + TCH]
                    sn = sin_sb[:, tl:tl + TCH]
                    # v staging evicts FIRST on ACT: the PE's next work (the
                    # v transposes) waits only on this copy, not on the five
                    # rope staging copies behind it.
                    vstage = rtmp.tile([H, TCH], bf16, tag="vstage", bufs=2)
                    nc.scalar.activation(vstage, g_ps[5], Copy)
                    # rope staging: every bank's copies are emitted before
                    # any rope math so banks free quickly for the next
                    # chunk's accumulation groups. Staging is 5-deep so no
                    # release ever queues behind another group's rope math.
                    rel = []
                    for g in range(5):
                        direct = rtmp.tile([H, TCH], f32, tag="rdir",
                                           bufs=5, name="direct")
                        swap = rtmp.tile([H, TCH], f32, tag="rswap",
                                         bufs=5, name="swap")
                        nc.scalar.activation(direct, g_ps[g], Copy)
                        nc.vector.tensor_copy(swap[0:HH, :], g_ps[g][HH:H, :])
                        nc.vector.tensor_copy(swap[HH:H, :], g_ps[g][0:HH, :])
                        rel.append((direct, swap))
                    # v: PE-transpose each [128,128] block into natural
                    # layout; ones column via memset (tiles are written once)
                    nc.vector.memset(vs[tch][:, :, H:H + 2], 0.0)
                    nc.vector.memset(vs[tch][:, :, H:H + 1], 1.0)
                    for j in range(NSUB):
                        tp = pv.tile([H, H], bf16, tag="vtp")
                        nc.tensor.transpose(
                            tp, vstage[:, j * H:(j + 1) * H], ident)
                        nc.vector.tensor_copy(vs[tch][:, j, 0:H], tp)
                    # the last two chunks' rope runs on the otherwise-idle
                    # GpSimd: their q/k are consumed late in phase 2, and
                    # keeping them off DVE means phase 2's first tri-masks
                    # and reciprocals don't queue behind a rope backlog.
                    eng = nc.gpsimd if tch >= NCH - 2 else nc.vector
                    rope_math(*rel[4], kTs[tch][0:HH, :], kTs[tch][HH:H, :],
                              cs, sn, eng)
                    for i in range(NHC):
                        rope_math(*rel[i], qTs[tch][0:HH, i, :],
                                  qTs[tch][HH:H, i, :], cs, sn, eng)

            # ---------------- phase 2: attention + o-projection -----------
            with ExitStack() as ph2:
                ppool = ph2.enter_context(tc.tile_pool(name="ppool", bufs=2))
                otpool = ph2.enter_context(tc.tile_pool(name="otpool", bufs=2))
                small = ph2.enter_context(tc.tile_pool(name="small", bufs=4))
                opool = ph2.enter_context(tc.tile_pool(name="opool", bufs=8))
                ps_s = ph2.enter_context(
                    tc.tile_pool(name="ps_s", bufs=3, space="PSUM"))
                ps_av = ph2.enter_context(
                    tc.tile_pool(name="ps_av", bufs=2, space="PSUM"))
                ps_o = ph2.enter_context(
                    tc.tile_pool(name="ps_o", bufs=2, space="PSUM"))
                ps_t = ph2.enter_context(
                    tc.tile_pool(name="ps_t", bufs=1, space="PSUM"))

                def oproj_group(b_, qc_, outT_prev, g):
                    # group g = u*8 + dc of the 32 o-projection groups for
                    # q-chunk (b_, qc_); eviction on DVE (ACT is saturated
                    # by exps while these interleave into the scores stage)
                    u, dc = divmod(g, 8)
                    trow = b_ * T + qc_ * TCH + u * H
                    ops = ps_o.tile([H, TCH], f32, tag="o")
                    for hh in range(NHC):
                        nc.tensor.matmul(
                            ops,
                            outT_prev[hh][u],
                            wo_sb[:, hh, dc * TCH:(dc + 1) * TCH],
                            start=(hh == 0), stop=(hh == NHC - 1),
                            skip_group_check=True)
                    o_sb = opool.tile([H, TCH], f32, tag="osb")
                    nc.vector.tensor_copy(o_sb, ops)
                    # split across two queues: a single [128,512] f32 write
                    # is 128 x 2KB descriptors on one queue (~8us latency),
                    # which stalled the o_sb/ps_o recycling chain
                    nc.sync.dma_start(
                        out=o_part[trow:trow + HH, dc * TCH:(dc + 1) * TCH],
                        in_=o_sb[0:HH, :])
                    nc.sync.dma_start(
                        out=o_part[trow + HH:trow + H,
                                   dc * TCH:(dc + 1) * TCH],
                        in_=o_sb[HH:H, :])

                def emit_transposes(tps):
                    # PE-transpose the previous head's normalized outputs to
                    # head-major; deferred into the next head's stage A so
                    # they never wait on the ACT normalize chain
                    for onat_, dst in tps:
                        tp = ps_t.tile([H, H], bf16, tag="otp")
                        nc.tensor.transpose(tp, onat_, ident)
                        nc.vector.tensor_copy(dst, tp)

                pending = None   # (b, qc, outT_tiles) awaiting o-projection
                pending_tp = []  # (onat, outT tile) awaiting PE transpose
                for b in range(B):
                    for qc in range(NTCH):
                        n_st = (qc + 1) * NSUB
                        outT_sb = [[otpool.tile([H, H], bf16,
                                                tag=f"ot{hh}_{uu}",
                                                name=f"ot{hh}_{uu}")
                                    for uu in range(NSUB)]
                                   for hh in range(NHC)]
                        for h in range(NHC):
                            rhs_q = qTs[NTCH * b + qc][:, h, :]
                            # stage A: scores + exp for all s-tiles, with the
                            # previous chunk's o-proj groups interleaved to
                            # keep the PE busy while ACT works through exps
                            fill_done = 0
                            pT2s = []
                            for st in range(n_st):
                                j = st - qc * NSUB   # >=0: diagonal band
                                c0 = max(j, 0) * H
                                sps = ps_s.tile([H, TCH], f32, tag="s")
                                pt = ppool.tile([H, TCH], bf16, tag=f"p{st}",
                                                name=f"p{st}")
                                pT2s.append(pt)
                                kt = kTs[NTCH * b + st // NSUB][
                                    :, (st % NSUB) * H:(st % NSUB + 1) * H]
                                nc.tensor.matmul(sps[:, c0:TCH], kt,
                                                 rhs_q[:, c0:TCH],
                                                 start=True, stop=True)
                                nc.scalar.activation(
                                    pt[:, c0:TCH], sps[:, c0:TCH],
                                    Exp, scale=C_SM)
                                if st == 0 and pending_tp:
                                    emit_transposes(pending_tp)
                                    pending_tp = []
                                if j >= 0:
                                    # in-place 0/1 triangle on the diagonal
                                    nc.vector.tensor_mul(
                                        pt[:, c0:c0 + H],
                                        pt[:, c0:c0 + H], tri)
                                if pending is not None:
                                    want = (st + 1) * 8 // n_st
                                    while fill_done < want:
                                        oproj_group(*pending[:3],
                                                    h * 8 + fill_done)
                                        fill_done += 1
                            if pending is not None and h == NHC - 1:
                                pending = None
                            # stage B: AV per 128-token subtile; the ones
                            # column of v accumulates the denominator into
                            # col 128 of the same PSUM group
                            for u in range(NSUB):
                                st_hi = min(n_st - 1, qc * NSUB + u)
                                avp = ps_av.tile([H, TCH], f32, tag="av")
                                for st in range(st_hi + 1):
                                    nc.tensor.matmul(
                                        avp[:, 0:H + 2],
                                        pT2s[st][:, u * H:(u + 1) * H],
                                        vs[NTCH * b + st // NSUB][
                                            :, st % NSUB, :],
                                        start=(st == 0), stop=(st == st_hi),
                                        skip_group_check=True)
                                recip = small.tile([H, 1], f32, tag="rc")
                                nc.vector.reciprocal(recip, avp[:, H:H + 1])
                                onat = small.tile([H, H], bf16, tag="on",
                                                  bufs=8)
                                nc.scalar.mul(onat, avp[:, 0:H], recip)
                                pending_tp.append((onat, outT_sb[h][u]))
                        pending = (b, qc, outT_sb)
                emit_transposes(pending_tp)
                pending_tp = []
                for g in range(32):
                    oproj_group(*pending[:3], g)

    nc.compile()
    return nc


_NC_CACHE = None


def kernel(x, wq, wk, wv, wo, positions):
    global _NC_CACHE
    import ml_dtypes
    from concourse.bass_utils import run_bass_kernel_spmd

    bf = ml_dtypes.bfloat16
    x = np.asarray(x, dtype=np.float32)
    positions = np.asarray(positions)

    xT = np.ascontiguousarray(x.reshape(TOK, D).T.astype(bf))
    wq_b = np.asarray(wq, dtype=np.float32).astype(bf)
    wk_b = np.asarray(wk, dtype=np.float32).astype(bf)
    wv_b = np.asarray(wv, dtype=np.float32).astype(bf)
    wo_b = np.asarray(wo, dtype=np.float32).astype(bf)

    # rope tables, transposed [H/2, T], duplicated across partition halves;
    # positions are identical across batches so one batch's worth suffices.
    fraction = 2.0 * np.arange(HH, dtype=np.float32) / H
    timescale = (THETA ** fraction).astype(np.float32)
    pos = positions.reshape(TOK)[:T].astype(np.float32)
    sinusoid = pos[None, :] / timescale[:, None]
    cosT = np.cos(sinusoid).astype(np.float32)
    sinT = np.sin(sinusoid).astype(np.float32)
    cosT = np.ascontiguousarray(np.concatenate([cosT, cosT], axis=0))
    sinT = np.ascontiguousarray(np.concatenate([sinT, sinT], axis=0))

    if _NC_CACHE is None:
        _NC_CACHE = _build_bass()
    nc = _NC_CACHE

    in_maps = []
    for c in range(NCORES):
        in_maps.append({
            "xT": xT,
            "wq": np.ascontiguousarray(wq_b[c * NHC:(c + 1) * NHC]),
            "wk": np.ascontiguousarray(wk_b[c]),
            "wv": np.ascontiguousarray(wv_b[c]),
            "wo": np.ascontiguousarray(wo_b[c * NHC:(c + 1) * NHC]),
            "cosT": cosT,
            "sinT": sinT,
        })

    trace = os.environ.get("BASS_KERNEL_TRACE", "0") == "1"
    res = run_bass_kernel_spmd(nc, in_maps, list(range(NCORES)), trace=trace)
    global LAST_RESULTS
    LAST_RESULTS = res
    out = np.zeros((TOK, D), dtype=np.float32)
    for c in range(NCORES):
        out += res.results[c]["o_part"]
    return out.reshape(B, T, D)


LAST_RESULTS = None


# revision 35
# speedup vs baseline: 1.6298x; 1.0453x over previous
"""GQA causal-attention prefill kernel for Trainium2, tensor-parallel over 8 NeuronCores.

Reference semantics: q/k/v projections + RoPE + causal GQA attention +
output projection, B=2, T=2048, D=4096, 32 q heads, 8 kv heads, head_dim 128.

Sharding: head-parallel. Core c gets q heads [4c, 4c+4), kv head c, and the
matching wo slice; each core computes a full-shape partial output
o_part = attn(heads of c) @ wo_c and the host sums the 8 partials.

v2 design (vs the 1250us baseline):
  - all matmul operands bf16 (same PE rate as fp32r at 512-wide, but full
    rate at ANY width, half the DMA/SBUF, and no walrus f32r-producer
    quirks). PSUM accumulation stays fp32; rope/softmax math in fp32.
  - ONE phase-1 sweep over all 8 token chunks (both batches): weights are
    loaded once (no per-batch reload) and no rope tail is exposed at a
    phase boundary.
  - weight DMAs are k-slice-interleaved with chunk-0 x tiles so the first
    projection matmul starts after ~1.5MB instead of ~15MB of DMA.
  - natural-layout AV with a ones column appended to v: one accumulation
    yields both sum_s p*v AND the softmax denominator l (column 128),
    removing the baseline's l-matmul + broadcast matmul (~96us of PE) and
    its [128,512] reciprocal/multiply DVE work. The denominator lands
    per-partition, so normalization fuses into the ACT eviction as a
    per-partition scale.
  - causal masking by construction: fully-masked 128-col blocks are never
    computed (scores matmuls cover only the valid column range; AV matmuls
    for fully-invalid blocks are skipped). Only the diagonal [128,128]
    triangles get an in-place 0/1 mask multiply on DVE.
  - transposes (v to natural layout, attention-out to head-major) run on
    the DMA XBAR (16-bit SBUF->SBUF transpose), not the PE.
  - o-projection groups of chunk N interleave into chunk N+1's scores/exp
    stage, where the PE would otherwise wait on ACT exps.
  - rope combine runs on DVE (bf16 writes), not the slow GpSimd.
"""

import os
import sys

sys.path.insert(0, "/opt/trn_rl_repo")

import numpy as np

B = 2
T = 2048
TOK = B * T
D = 4096
NQ = 32
NKV = 8
H = 128
HH = H // 2
THETA = 10000.0
NCORES = 8
NHC = NQ // NCORES          # q heads per core (4)
KPC = D // H                # contraction chunks of 128 over D (32)
TCH = 512                   # token chunk
NCH = TOK // TCH            # 8 chunks across both batches
NTCH = T // TCH             # 4 chunks per batch
NSUB = TCH // H             # 4 128-token subtiles per chunk
C_SM = 1.0 / np.sqrt(H)     # softmax scale


def _build_bass():
    import concourse.bacc as bacc
    import concourse.mybir as mybir
    import concourse.tile as tile
    from concourse.masks import make_identity

    f32 = mybir.dt.float32
    bf16 = mybir.dt.bfloat16
    Exp = mybir.ActivationFunctionType.Exp
    Copy = mybir.ActivationFunctionType.Copy

    nc = bacc.Bacc("TRN2", target_bir_lowering=False, debug=False,
                   num_devices=NCORES)

    # weights arrive pre-transposed to partition-major layouts so every
    # weight DMA moves multi-KB contiguous runs per partition row instead
    # of thousands of 256B descriptors
    xT = nc.declare_dram_parameter("xT", [D, TOK], bf16, isOutput=False)
    wq = nc.declare_dram_parameter("wq", [H, NHC, KPC, H], bf16,
                                   isOutput=False)
    wk = nc.declare_dram_parameter("wk", [H, KPC, H], bf16, isOutput=False)
    wv = nc.declare_dram_parameter("wv", [H, KPC, H], bf16, isOutput=False)
    wo = nc.declare_dram_parameter("wo", [H, NHC, D], bf16, isOutput=False)
    # rope tables duplicated across both partition halves (row p and p+64
    # hold the same values); one batch's worth - positions are identical
    # across batches.
    cosT = nc.declare_dram_parameter("cosT", [H, T], f32, isOutput=False)
    sinT = nc.declare_dram_parameter("sinT", [H, T], f32, isOutput=False)
    o_part = nc.declare_dram_parameter("o_part", [TOK, D], f32, isOutput=True)

    with tile.TileContext(nc) as tc:
        from contextlib import ExitStack

        with ExitStack() as top:
            consts = top.enter_context(tc.tile_pool(name="consts", bufs=1))
            # bf16 identity for PE transposes of bf16 tiles
            ident_f32 = consts.tile([H, H], f32, tag="idf")
            make_identity(nc, ident_f32)
            ident = consts.tile([H, H], bf16, tag="id")
            nc.vector.tensor_copy(ident, ident_f32)

            # persistent activations for both batches
            act = top.enter_context(tc.tile_pool(name="act", bufs=1))
            qTs = [act.tile([H, NHC, TCH], bf16, tag=f"qT{i}", name=f"qT{i}")
                   for i in range(NCH)]
            kTs = [act.tile([H, TCH], bf16, tag=f"kT{i}", name=f"kT{i}")
                   for i in range(NCH)]
            # v natural [s, j, col]: col 0:128 = v, col 128 = 1.0 (the ones
            # column that accumulates the softmax denominator in AV),
            # col 129 = 0 pad for 4-byte row alignment
            vs = [act.tile([H, NSUB, H + 2], bf16, tag=f"v{i}", name=f"v{i}")
                  for i in range(NCH)]
            wopool = top.enter_context(tc.tile_pool(name="wopool", bufs=1))
            wo_sb = wopool.tile([H, NHC, D], bf16, tag="wo")

            # ---------------- phase 1: projections + rope, one sweep ------
            with ExitStack() as ph1:
                wpool = ph1.enter_context(tc.tile_pool(name="wpool", bufs=1))
                xpool = ph1.enter_context(tc.tile_pool(name="xpool", bufs=8))
                rtmp = ph1.enter_context(tc.tile_pool(name="rtmp", bufs=2))
                pj = ph1.enter_context(
                    tc.tile_pool(name="pj", bufs=1, space="PSUM"))
                pv = ph1.enter_context(
                    tc.tile_pool(name="pv", bufs=2, space="PSUM"))

                wqs = [wpool.tile([H, KPC, H], bf16, tag=f"wq{i}",
                                  name=f"wq{i}") for i in range(NHC)]
                wk_sb = wpool.tile([H, KPC, H], bf16, tag="wk")
                wv_sb = wpool.tile([H, KPC, H], bf16, tag="wv")
                cos_sb = wpool.tile([H, T], f32, tag="cos")
                sin_sb = wpool.tile([H, T], f32, tag="sin")
                # k-slice-interleaved weight+x loads: the first matmul needs
                # only wq0 slice c8=0 and x chunk-0 k=0, so issue those first
                x_c0 = []
                for c8 in range(4):
                    sl = slice(c8 * 8, (c8 + 1) * 8)
                    for i in range(NHC):
                        nc.sync.dma_start(out=wqs[i][:, sl, :],
                                          in_=wq[:, i, sl, :])
                    nc.sync.dma_start(out=wk_sb[:, sl, :], in_=wk[:, sl, :])
                    nc.sync.dma_start(out=wv_sb[:, sl, :], in_=wv[:, sl, :])
                    for k in range(c8 * 8, (c8 + 1) * 8):
                        x_t = xpool.tile([H, TCH], bf16, tag="x")
                        nc.sync.dma_start(out=x_t,
                                          in_=xT[k * H:(k + 1) * H, 0:TCH])
                        x_c0.append(x_t)
                nc.sync.dma_start(out=cos_sb, in_=cosT[:, 0:T])
                nc.sync.dma_start(out=sin_sb, in_=sinT[:, 0:T])

                def rope_math(direct, swap, dst_first, dst_second, cs, sn,
                              eng):
                    # q'[0:64] = q[0:64]*cos - q[64:]*sin
                    # q'[64:]  = q[64:]*cos + q[0:64]*sin
                    # The rope tables are duplicated across partition halves
                    # precisely so both muls run as single [128,512] ops.
                    # (both-SBUF operand pairs must share a base partition,
                    # hence the half-swapped staging copy `swap`.)
                    tmp = rtmp.tile([H, TCH], f32, tag="rt", bufs=2)
                    tmp2 = rtmp.tile([H, TCH], f32, tag="rt2", bufs=2)
                    eng.tensor_mul(tmp, swap, sn)
                    eng.tensor_mul(tmp2, direct, cs)
                    eng.tensor_sub(dst_first, tmp2[0:HH, :], tmp[0:HH, :])
                    eng.tensor_add(dst_second, tmp2[HH:H, :], tmp[HH:H, :])

                last = KPC - 1
                # chunk order: the two chunks roped last (4, 5) are batch
                # 1's earliest q-chunks, consumed ~150us into phase 2 --
                # so the DVE rope backlog at the phase boundary never gates
                # phase-2 work
                for tch in [0, 1, 2, 3, 6, 7, 4, 5]:
                    t0 = tch * TCH
                    tl = (tch % NTCH) * TCH      # token offset within batch
                    g_ps = [pj.tile([H, TCH], f32, tag=f"g{i}",
                                    name=f"g_ps{i}") for i in range(6)]
                    for k in range(KPC):
                        if tch == 0:
                            x_t = x_c0[k]
                        else:
                            x_t = xpool.tile([H, TCH], bf16, tag="x")
                            nc.sync.dma_start(
                                out=x_t,
                                in_=xT[k * H:(k + 1) * H, t0:t0 + TCH])
                        lhs = [wqs[0][:, k, :], wqs[1][:, k, :],
                               wqs[2][:, k, :], wqs[3][:, k, :],
                               wk_sb[:, k, :], wv_sb[:, k, :]]
                        for i in range(6):
                            nc.tensor.matmul(
                                g_ps[i], lhs[i], x_t,
                                start=(k == 0), stop=(k == last),
                                skip_group_check=True)
                    # wo prefetch once the DMA queues have drained the bulk
                    # of the weight traffic; lands well before phase 2.
                    if tch == 7:
                        for dc8 in range(8):
                            sl = slice(dc8 * TCH, (dc8 + 1) * TCH)
                            nc.sync.dma_start(out=wo_sb[:, :, sl],
                                              in_=wo[:, :, sl])
                    cs = cos_sb[:, tl:tl + TCH]
                    sn = sin_sb[:, tl:tl + TCH]
                    # v staging evicts FIRST on ACT: the PE's next work (the
                    # v transposes) waits only on this copy, not on the five
                    # rope staging copies behind it.
                    vstage = rtmp.tile([H, TCH], bf16, tag="vstage", bufs=2)
                    nc.scalar.activation(vstage, g_ps[5], Copy)
                    # rope staging: every bank's copies are emitted before
                    # any rope math so banks free quickly for the next
                    # chunk's accumulation groups. Staging is 5-deep so no
                    # release ever queues behind another group's rope math.
                    rel = []
                    for g in range(5):
                        direct = rtmp.tile([H, TCH], f32, tag="rdir",
                                           bufs=5, name="direct")
                        swap = rtmp.tile([H, TCH], f32, tag="rswap",
                                         bufs=5, name="swap")
                        nc.scalar.activation(direct, g_ps[g], Copy)
                        nc.vector.tensor_copy(swap[0:HH, :], g_ps[g][HH:H, :])
                        nc.vector.tensor_copy(swap[HH:H, :], g_ps[g][0:HH, :])
                        rel.append((direct, swap))
                    # v: PE-transpose each [128,128] block into natural
                    # layout; ones column via memset (tiles are written once)
                    nc.vector.memset(vs[tch][:, :, H:H + 2], 0.0)
                    nc.vector.memset(vs[tch][:, :, H:H + 1], 1.0)
                    for j in range(NSUB):
                        tp = pv.tile([H, H], bf16, tag="vtp")
                        nc.tensor.transpose(
                            tp, vstage[:, j * H:(j + 1) * H], ident)
                        nc.vector.tensor_copy(vs[tch][:, j, 0:H], tp)
                    rope_math(*rel[4], kTs[tch][0:HH, :], kTs[tch][HH:H, :],
                              cs, sn, nc.vector)
                    for i in range(NHC):
                        rope_math(*rel[i], qTs[tch][0:HH, i, :],
                                  qTs[tch][HH:H, i, :], cs, sn, nc.vector)

            # ---------------- phase 2: attention + o-projection -----------
            with ExitStack() as ph2:
                ppool = ph2.enter_context(tc.tile_pool(name="ppool", bufs=2))
                otpool = ph2.enter_context(tc.tile_pool(name="otpool", bufs=2))
                small = ph2.enter_context(tc.tile_pool(name="small", bufs=4))
                opool = ph2.enter_context(tc.tile_pool(name="opool", bufs=8))
                ps_s = ph2.enter_context(
                    tc.tile_pool(name="ps_s", bufs=3, space="PSUM"))
                ps_av = ph2.enter_context(
                    tc.tile_pool(name="ps_av", bufs=2, space="PSUM"))
                ps_o = ph2.enter_context(
                    tc.tile_pool(name="ps_o", bufs=2, space="PSUM"))
                ps_t = ph2.enter_context(
                    tc.tile_pool(name="ps_t", bufs=1, space="PSUM"))

                def oproj_group(b_, qc_, outT_prev, g):
                    # group g = u*8 + dc of the 32 o-projection groups for
                    # q-chunk (b_, qc_); eviction on DVE (ACT is saturated
                    # by exps while these interleave into the scores stage)
                    u, dc = divmod(g, 8)
                    trow = b_ * T + qc_ * TCH + u * H
                    ops = ps_o.tile([H, TCH], f32, tag="o")
                    for hh in range(NHC):
                        nc.tensor.matmul(
                            ops,
                            outT_prev[hh][u],
                            wo_sb[:, hh, dc * TCH:(dc + 1) * TCH],
                            start=(hh == 0), stop=(hh == NHC - 1),
                            skip_group_check=True)
                    o_sb = opool.tile([H, TCH], f32, tag="osb")
                    nc.vector.tensor_copy(o_sb, ops)
                    # split across two queues: a single [128,512] f32 write
                    # is 128 x 2KB descriptors on one queue (~8us latency),
                    # which stalled the o_sb/ps_o recycling chain
                    nc.sync.dma_start(
                        out=o_part[trow:trow + HH, dc * TCH:(dc + 1) * TCH],
                        in_=o_sb[0:HH, :])
                    nc.sync.dma_start(
                        out=o_part[trow + HH:trow + H,
                                   dc * TCH:(dc + 1) * TCH],
                        in_=o_sb[HH:H, :])

                def emit_transposes(tps):
                    # PE-transpose the previous head's normalized outputs to
                    # head-major; deferred into the next head's stage A so
                    # they never wait on the ACT normalize chain
                    for onat_, dst in tps:
                        tp = ps_t.tile([H, H], bf16, tag="otp")
                        nc.tensor.transpose(tp, onat_, ident)
                        nc.vector.tensor_copy(dst, tp)

                pending = None   # (b, qc, outT_tiles) awaiting o-projection
                pending_tp = []  # (onat, outT tile) awaiting PE transpose
                for b in range(B):
                    for qc in range(NTCH):
                        n_st = (qc + 1) * NSUB
                        outT_sb = [[otpool.tile([H, H], bf16,
                                                tag=f"ot{hh}_{uu}",
                                                name=f"ot{hh}_{uu}")
                                    for uu in range(NSUB)]
                                   for hh in range(NHC)]
                        for h in range(NHC):
                            rhs_q = qTs[NTCH * b + qc][:, h, :]
                            # stage A: scores + exp for all s-tiles, with the
                            # previous chunk's o-proj groups interleaved to
                            # keep the PE busy while ACT works through exps
                            fill_done = 0
                            pT2s = []
                            for st in range(n_st):
                                j = st - qc * NSUB   # >=0: diagonal band
                                c0 = max(j, 0) * H
                                sps = ps_s.tile([H, TCH], f32, tag="s")
                                pt = ppool.tile([H, TCH], bf16, tag=f"p{st}",
                                                name=f"p{st}")
                                pT2s.append(pt)
                                kt = kTs[NTCH * b + st // NSUB][
                                    :, (st % NSUB) * H:(st % NSUB + 1) * H]
                                nc.tensor.matmul(sps[:, c0:TCH], kt,
                                                 rhs_q[:, c0:TCH],
                                                 start=True, stop=True)
                                nc.scalar.activation(
                                    pt[:, c0:TCH], sps[:, c0:TCH],
                                    Exp, scale=C_SM)
                                if st == 0 and pending_tp:
                                    emit_transposes(pending_tp)
                                    pending_tp = []
                                if j >= 0:
                                    # causal triangle on the diagonal block,
                                    # in place on the otherwise-idle GpSimd
                                    # (keeps stage A free of DVE deps):
                                    # keep p[s, c] where c >= s, else 0
                                    nc.gpsimd.affine_select(
                                        out=pt[:, c0:c0 + H],
                                        in_=pt[:, c0:c0 + H],
                                        compare_op=mybir.AluOpType.is_ge,
                                        fill=0.0, base=0,
                                        pattern=[[1, H]],
                                        channel_multiplier=-1,
                                    )
                                if pending is not None:
                                    want = (st + 1) * 8 // n_st
                                    while fill_done < want:
                                        oproj_group(*pending[:3],
                                                    h * 8 + fill_done)
                                        fill_done += 1
                            if pending is not None and h == NHC - 1:
                                pending = None
                            # stage B: AV per 128-token subtile; the ones
                            # column of v accumulates the denominator into
                            # col 128 of the same PSUM group
                            for u in range(NSUB):
                                st_hi = min(n_st - 1, qc * NSUB + u)
                                avp = ps_av.tile([H, TCH], f32, tag="av")
                                for st in range(st_hi + 1):
                                    nc.tensor.matmul(
                                        avp[:, 0:H + 2],
                                        pT2s[st][:, u * H:(u + 1) * H],
                                        vs[NTCH * b + st // NSUB][
                                            :, st % NSUB, :],
                                        start=(st == 0), stop=(st == st_hi),
                                        skip_group_check=True)
                                recip = small.tile([H, 1], f32, tag="rc")
                                nc.vector.reciprocal(recip, avp[:, H:H + 1])
                                onat = small.tile([H, H], bf16, tag="on",
                                                  bufs=8)
                                nc.scalar.mul(onat, avp[:, 0:H], recip)
                                pending_tp.append((onat, outT_sb[h][u]))
                        pending = (b, qc, outT_sb)
                emit_transposes(pending_tp)
                pending_tp = []
                for g in range(32):
                    oproj_group(*pending[:3], g)

    nc.compile()
    return nc


_NC_CACHE = None


def host_prep(x, wq, wk, wv, wo, positions):
    import ml_dtypes

    bf = ml_dtypes.bfloat16
    x = np.asarray(x, dtype=np.float32)
    positions = np.asarray(positions)

    xT = np.ascontiguousarray(x.reshape(TOK, D).T.astype(bf))
    # partition-major weight layouts (see kernel comment):
    #   wq [NQ, D, H]  -> per-core [128p, NHC, KPC, H]
    #   wk/wv [NKV, D, H] -> per-core [128p, KPC, H]
    #   wo [NQ, H, D]  -> per-core [128p, NHC, D]
    wq_b = np.ascontiguousarray(
        np.asarray(wq, np.float32).astype(bf)
        .reshape(NQ, KPC, H, H).transpose(0, 2, 1, 3))    # [NQ, p, c, m]
    wk_b = np.ascontiguousarray(
        np.asarray(wk, np.float32).astype(bf)
        .reshape(NKV, KPC, H, H).transpose(0, 2, 1, 3))   # [NKV, p, c, m]
    wv_b = np.ascontiguousarray(
        np.asarray(wv, np.float32).astype(bf)
        .reshape(NKV, KPC, H, H).transpose(0, 2, 1, 3))
    wo_b = np.asarray(wo, np.float32).astype(bf)          # [NQ, H(p), D]

    # rope tables, transposed [H/2, T], duplicated across partition halves;
    # positions are identical across batches so one batch's worth suffices.
    fraction = 2.0 * np.arange(HH, dtype=np.float32) / H
    timescale = (THETA ** fraction).astype(np.float32)
    pos = positions.reshape(TOK)[:T].astype(np.float32)
    sinusoid = pos[None, :] / timescale[:, None]
    cosT = np.cos(sinusoid).astype(np.float32)
    sinT = np.sin(sinusoid).astype(np.float32)
    cosT = np.ascontiguousarray(np.concatenate([cosT, cosT], axis=0))
    sinT = np.ascontiguousarray(np.concatenate([sinT, sinT], axis=0))

    in_maps = []
    for c in range(NCORES):
        hs = slice(c * NHC, (c + 1) * NHC)
        in_maps.append({
            "xT": xT,
            # [p, NHC, KPC, H] / [p, KPC, H] / [p, NHC, D]
            "wq": np.ascontiguousarray(wq_b[hs].transpose(1, 0, 2, 3)),
            "wk": np.ascontiguousarray(wk_b[c]),
            "wv": np.ascontiguousarray(wv_b[c]),
            "wo": np.ascontiguousarray(wo_b[hs].transpose(1, 0, 2)),
            "cosT": cosT,
            "sinT": sinT,
        })
    return in_maps


def kernel(x, wq, wk, wv, wo, positions):
    global _NC_CACHE
    from concourse.bass_utils import run_bass_kernel_spmd

    in_maps = host_prep(x, wq, wk, wv, wo, positions)
    if _NC_CACHE is None:
        _NC_CACHE = _build_bass()
    nc = _NC_CACHE

    trace = os.environ.get("BASS_KERNEL_TRACE", "0") == "1"
    res = run_bass_kernel_spmd(nc, in_maps, list(range(NCORES)), trace=trace)
    global LAST_RESULTS
    LAST_RESULTS = res
    out = np.zeros((TOK, D), dtype=np.float32)
    for c in range(NCORES):
        out += res.results[c]["o_part"]
    return out.reshape(B, T, D)


LAST_RESULTS = None
